# revision 1
# baseline (speedup 1.0000x reference)
"""PointPillarsScatter Trainium2 Bass kernel (8-core SPMD, data parallel).

Problem: scatter M=100000 pillar feature rows (C=64, fp32) into a
(B=4, C=64, NY=512, NX=512) canvas addressed by (batch, y, x)
coordinates. Duplicate coordinates resolve last-write-wins (matching
XLA CPU scatter .set; the neuron-backend reference is nondeterministic
under collisions, run-to-run noise ~1e-2 relative).

Sharding (data-parallel, no cross-core communication): core k owns
batch b = k//2 and y-half yh = k%2 — a (64, 256, 512) output slice =
131072 cells. Cells are processed as 65536 PAIRS (cell c, cell
c+65536), i.e. output rows y and y+128, so each gathered/scattered
element is 512B (full DMA descriptor rate) and each PE transpose
handles a full [128, 128] block.

Per-core device pipeline (16 regions x 4096 pair-slots):
- DVE/Pool memset two canvas tiles own/peer [128, 16, 128] f32 (a
  pair-slot s maps to partition s%128, free group s//256, tile chosen
  by parity (s//128)&1 — the hardware layout of SBUF-dst
  dma_scatter_add with sbuf_tokens_per_rank=128).
- One DMA loads the region's host-packed non-empty pair rows (wrapped
  [128, 6, 128] layout), one dma_scatter_add (CCE, fp32 add onto the
  zeroed tiles == placement) scatters them to their slots. Only
  ~700 of 4096 slots are non-empty, so this moves ~6x less data than
  gathering every cell. Padding descriptors are pointed at EMPTY
  slots: concurrent CCE read-modify-writes racing on one occupied
  address can drop a real pillar's add.
- 32 PE transpose-mode matmuls ([128 pairs, 128] -> [128, 128]; out
  partitions = channels of the A half (0:64) and B half (64:128))
  produce channel-major data in PSUM.
- scalar/vector engines copy PSUM -> SBUF, two DMAs per half-region
  write output rows y..y+3 and y+128..y+131 ([64 partitions, 8KB
  contiguous per channel] descriptors).

SWDGE note: one Pool-DMA instruction can carry at most ~1024
descriptors (default dynamic-DMA ring), hence num_idxs <= 1024.
"""

import sys

import numpy as np

_TRN_REPO = "/opt/trn_rl_repo"
if _TRN_REPO not in sys.path:
    sys.path.insert(0, _TRN_REPO)

NY, NX, C, B = 512, 512, 64, 4
CELLS = B * NY * NX             # 1048576
N_CORES = 8
CORE_CELLS = CELLS // N_CORES   # 131072
HALF = CORE_CELLS // 2          # 65536 pairs per core
REGIONS = 16
REGION_PAIRS = HALF // REGIONS  # 4096
MAX_NE = 768                    # scatter rows per region (observed max 740)
MAX_NE_FALLBACK = 1024          # recompile capacity if inputs ever differ
PAIR_ELEM = 2 * C               # 128 f32 = 512B


def build_nc(max_ne=MAX_NE):
    """Build the per-core Bass program (SPMD: same NEFF on all 8 cores)."""
    from concourse import bacc, masks, tile
    from concourse import mybir

    f32 = mybir.dt.float32
    i16 = mybir.dt.int16

    nc = bacc.Bacc(
        "TRN2", target_bir_lowering=False, debug=False, num_devices=N_CORES
    )
    table = nc.dram_tensor(
        "table", [REGIONS, max_ne, PAIR_ELEM], f32, kind="ExternalInput"
    )
    idx = nc.dram_tensor(
        "idx", [128, REGIONS * (max_ne // 16)], i16, kind="ExternalInput"
    )
    out = nc.dram_tensor("out", [C, CORE_CELLS], f32, kind="ExternalOutput")

    ncols = max_ne // 16

    with tile.TileContext(nc) as tc:
        with (
            tc.tile_pool(name="const", bufs=1) as cpool,
            tc.tile_pool(name="canvas", bufs=6) as canvas_pool,
            tc.tile_pool(name="srcp", bufs=6) as spool,
            tc.tile_pool(name="outp", bufs=6) as opool,
            tc.tile_pool(name="psum", bufs=2, space="PSUM") as ppool,
        ):
            ident = cpool.tile([128, 128], f32)
            masks.make_identity(nc, ident[:])
            idx_sb = cpool.tile([128, REGIONS * ncols], i16)
            nc.sync.dma_start(out=idx_sb[:], in_=idx[:])

            for g in range(REGIONS):
                own = canvas_pool.tile([128, REGIONS, PAIR_ELEM], f32, tag="own")
                peer = canvas_pool.tile([128, REGIONS, PAIR_ELEM], f32, tag="peer")
                nc.vector.memset(own[:], 0.0)
                nc.gpsimd.memset(peer[:], 0.0)

                src = spool.tile([128, max_ne // 128, PAIR_ELEM], f32)
                nc.sync.dma_start(
                    out=src[:], in_=table[g].rearrange("(c p) e -> p c e", p=128)
                )

                nc.gpsimd.dma_scatter_add(
                    out_ap=own[:],
                    in_ap=src[:],
                    idxs_ap=idx_sb[:, g * ncols:(g + 1) * ncols],
                    num_idxs=max_ne,
                    num_idxs_reg=max_ne,
                    elem_size=PAIR_ELEM,
                    parity_reg=0,
                    out_ap_other=peer[:],
                    sbuf_tokens_per_rank=128,
                )

                for h in range(2):
                    ps = ppool.tile([128, 2048], f32)
                    for jj in range(16):
                        j = 16 * h + jj
                        blk = (own if j % 2 == 0 else peer)[:, j // 2, :]
                        nc.tensor.transpose(
                            ps[:, 128 * jj:128 * (jj + 1)], blk, ident[:]
                        )
                    ot = opool.tile([128, 2048], f32)
                    if (2 * g + h) % 2 == 0:
                        nc.scalar.copy(ot[:], ps[:])
                    else:
                        nc.vector.tensor_copy(ot[:], ps[:])
                    # alternate output DMAs across the two HWDGE queues
                    # (SP / Activation) so descriptor generation pipelines
                    eng = nc.sync if h % 2 == 0 else nc.scalar
                    start_a = g * REGION_PAIRS + h * 2048
                    eng.dma_start(
                        out=out[0:C, start_a:start_a + 2048], in_=ot[0:C, :]
                    )
                    eng.dma_start(
                        out=out[0:C, HALF + start_a:HALF + start_a + 2048],
                        in_=ot[C:2 * C, :],
                    )
    nc.compile()
    return nc


def host_prep(pillar_features, coordinates, max_ne):
    """Per-core {table, idx} maps. Last write wins on duplicate cells."""
    pf = np.ascontiguousarray(np.asarray(pillar_features), dtype=np.float32)
    coords = np.asarray(coordinates)
    m = pf.shape[0]
    flat = (
        coords[:, 0].astype(np.int64) * (NY * NX)
        + coords[:, 2].astype(np.int64) * NX
        + coords[:, 3].astype(np.int64)
    )
    order = np.argsort(flat, kind="stable")
    fs = flat[order]
    is_last = np.empty(m, dtype=bool)
    if m > 1:
        is_last[:-1] = fs[:-1] != fs[1:]
    is_last[-1] = True
    occ = np.full(CELLS, -1, dtype=np.int64)
    occ[fs[is_last]] = order[is_last]

    in_maps = []
    for k in range(N_CORES):
        b, yh = k // 2, k % 2
        base = b * (NY * NX) + yh * (NY // 2) * NX
        occ_k = occ[base: base + CORE_CELLS]
        p_a, p_b = occ_k[:HALF], occ_k[HALF:]

        tbl = np.zeros((REGIONS, max_ne, PAIR_ELEM), dtype=np.float32)
        idx_all = np.zeros((REGIONS, max_ne), dtype=np.int16)
        for g in range(REGIONS):
            sl = slice(g * REGION_PAIRS, (g + 1) * REGION_PAIRS)
            ra, rb = p_a[sl], p_b[sl]
            ne = np.where((ra >= 0) | (rb >= 0))[0]
            n = len(ne)
            if n > max_ne:
                return None  # caller retries with larger capacity
            m_a = ra[ne] >= 0
            m_b = rb[ne] >= 0
            tbl[g, :n][m_a, :C] = pf[ra[ne][m_a]]
            tbl[g, :n][m_b, C:] = pf[rb[ne][m_b]]
            idx_all[g, :n] = ne.astype(np.int16)
            # padding rows add zeros; target only EMPTY slots (a racing
            # CCE read-modify-write on an occupied slot can drop data)
            empty = np.setdiff1d(
                np.arange(REGION_PAIRS, dtype=np.int16),
                ne.astype(np.int16),
                assume_unique=True,
            )
            assert len(empty) > 0
            idx_all[g, n:] = np.resize(empty, max_ne - n)

        blk = idx_all.reshape(REGIONS, max_ne // 16, 16)
        blk = blk.transpose(2, 0, 1).reshape(16, REGIONS * (max_ne // 16))
        idx_tile = np.ascontiguousarray(np.tile(blk, (8, 1)))
        in_maps.append({"table": tbl, "idx": idx_tile})
    return in_maps


_NC_CACHE = {}


def _get_nc(max_ne):
    if max_ne not in _NC_CACHE:
        _NC_CACHE[max_ne] = build_nc(max_ne)
    return _NC_CACHE[max_ne]


def kernel(pillar_features, coordinates, batch_size):
    assert int(batch_size) == B
    from concourse.bass_utils import run_bass_kernel_spmd

    in_maps = host_prep(pillar_features, coordinates, MAX_NE)
    max_ne = MAX_NE
    if in_maps is None:
        max_ne = MAX_NE_FALLBACK
        in_maps = host_prep(pillar_features, coordinates, max_ne)
        assert in_maps is not None, "region occupancy exceeds fallback capacity"
    nc = _get_nc(max_ne)
    res = run_bass_kernel_spmd(nc, in_maps, list(range(N_CORES)))

    full = np.empty((B, C, NY, NX), dtype=np.float32)
    for k in range(N_CORES):
        b, yh = k // 2, k % 2
        out_k = res.results[k]["out"].reshape(C, NY // 2, NX)
        full[b, :, yh * (NY // 2):(yh + 1) * (NY // 2), :] = out_k
    return full



# revision 16
# speedup vs baseline: 2.1377x; 2.1377x over previous
"""PointPillarsScatter Trainium2 Bass kernel (8-core SPMD, data parallel).

Problem: scatter M=100000 pillar feature rows (C=64, fp32) into a
(B=4, C=64, NY=512, NX=512) canvas addressed by (batch, y, x)
coordinates. Duplicate coordinates resolve last-write-wins (matching
XLA CPU scatter .set; the neuron-backend reference is nondeterministic
under collisions, run-to-run noise ~1e-2 relative).

Sharding (data-parallel, no cross-core communication): core k owns
batch b = k//2 and y-half yh = k%2 — a (64, 256, 512) output slice =
131072 cells, processed as 65536 PAIR-COLUMNS (cell c, cell c+65536).

Device algorithm (one-hot matmul placement, all-bf16 data path):
the canvas is produced directly by PE matmuls out[pairpos, col] =
sum_r tbl[r, pairpos] * P[r, col], where tbl rows are the host-packed
occupied pair rows (128 bf16 = cellA ch0..63 | cellB ch0..63) of a
512-paircol group and P is the one-hot placement matrix built on DVE
via is_equal(iota, colidx[r]).  Empty columns get all-zero P columns,
so the matmul also materializes the zeros — no canvas memset, no
SBUF scatter pass, and the only DMA traffic is the packed table read
(~4.2MB/core) plus the bf16 output write (16.8MB/core), vs 45MB/core
for the memset+scatter+transpose pipeline. bf16 quantization adds
~0.2% relative error, far under the 2e-2 gate (the fp32 reference's
own scatter-collision nondeterminism is already ~0.8%).

Per core: 128 groups x 512 paircols, capacity 128 rows/group (observed
max occupancy 108). 16 batches x 8 groups; per batch: one 256KB table
load, 8 P-builds (DVE), 8 matmuls (PE, bf16, 512 free columns each),
2 PSUM->SBUF bf16 copies (ACT/DVE balanced), 2 x 512KB output DMAs.
"""

import sys

import numpy as np
import ml_dtypes

_TRN_REPO = "/opt/trn_rl_repo"
if _TRN_REPO not in sys.path:
    sys.path.insert(0, _TRN_REPO)

NY, NX, C, B = 512, 512, 64, 4
CELLS = B * NY * NX             # 1048576
N_CORES = 8
CORE_CELLS = CELLS // N_CORES   # 131072
HALF = CORE_CELLS // 2          # 65536 paircols per core
GCOLS = 512                     # paircols per group (one matmul)
NGROUPS = HALF // GCOLS         # 128
CAP = 112                       # row capacity per group (observed max 108)
GPB = 8                         # groups per batch
NB = NGROUPS // GPB             # 16 batches
GCOLS_FB = 256                  # fallback: smaller groups if CAP overflows


def build_nc(gcols=GCOLS):
    """Build the per-core Bass program (SPMD: same NEFF on all 8 cores)."""
    from concourse import bacc, tile
    from concourse import mybir

    f32 = mybir.dt.float32
    bf16 = mybir.dt.bfloat16
    i16 = mybir.dt.int16

    ngroups = HALF // gcols
    nb = ngroups // GPB
    bcols = GPB * gcols         # paircols per batch (4096)

    nc = bacc.Bacc(
        "TRN2", target_bir_lowering=False, debug=False, num_devices=N_CORES
    )
    table = nc.dram_tensor(
        "table", [nb, CAP, GPB, 128], bf16, kind="ExternalInput"
    )
    colidx = nc.dram_tensor("colidx", [128, ngroups], f32, kind="ExternalInput")
    iota_t = nc.dram_tensor("iota_t", [128, gcols], i16, kind="ExternalInput")
    out = nc.dram_tensor("out", [C, CORE_CELLS], bf16, kind="ExternalOutput")

    with tile.TileContext(nc) as tc:
        with (
            tc.tile_pool(name="const", bufs=1) as cpool,
            tc.tile_pool(name="tbl", bufs=8) as tpool,
            tc.tile_pool(name="pmat", bufs=12) as ppool,
            tc.tile_pool(name="stage", bufs=4) as spool,
            tc.tile_pool(name="psum", bufs=4, space="PSUM") as psum_pool,
        ):
            iota_sb = cpool.tile([128, gcols], i16)
            nc.scalar.dma_start(out=iota_sb[:], in_=iota_t[:])
            colidx_sb = cpool.tile([128, ngroups], f32)
            nc.scalar.dma_start(out=colidx_sb[:], in_=colidx[:])

            # PE p-state warm-up source tile (see first-batch matmuls):
            # the ramp to full PE clock takes 3us of continuous busy
            warm = ppool.tile([128, gcols], bf16)
            nc.gpsimd.memset(warm[:], 0.0)

            # output viewed as [half, chan, col] so one DMA covers both the
            # upper (cell c) and lower (cell c+HALF) channel blocks:
            # stage partition p maps to (half=p//64, chan=p%64)
            out_v = out.rearrange("c (h x) -> h c x", h=2)

            def issue_out_dmas(b, stage):
                # two-batch-delayed output DMAs: the stage data is long
                # copied, so these never park at the SP sequencer head
                # (a sem wait there would stall every later DMA dispatch)
                for q in range(2):
                    w = 4 * gcols
                    s0 = q * w
                    c0 = b * bcols + s0
                    nc.sync.dma_start(
                        out=out_v[:, :, c0:c0 + w], in_=stage[:, s0:s0 + w]
                    )

            pending = []
            for b in range(nb):
                tbl_t = tpool.tile([CAP, GPB, 128], bf16)
                nc.sync.dma_start(out=tbl_t[:], in_=table[b])
                stage = spool.tile([128, bcols], bf16)
                # all 8 P-builds up front: the single DVE copy below then
                # never head-of-line-blocks the next batch's P chain
                Ps = []
                for gq in range(GPB):
                    g = b * GPB + gq
                    P = ppool.tile([128, gcols], bf16)
                    # one-hot placement: P[r, c] = (c == colidx[r, g])
                    nc.vector.tensor_scalar(
                        P[:], iota_sb[:], colidx_sb[:, g:g + 1], None,
                        op0=mybir.AluOpType.is_equal,
                    )
                    Ps.append(P)
                for h in range(4):
                    # 2 groups per PSUM tile (2 banks) -> 4 in-flight tiles,
                    # so matmuls never stall on a not-yet-copied PSUM bank
                    ps = psum_pool.tile([128, 2 * gcols], f32)
                    if b == 0 and h == 0:
                        # warm-up matmuls; the first real matmul's start=True
                        # PSUM reset overwrites their garbage (same engine,
                        # in-order)
                        for _ in range(6):
                            nc.tensor.matmul(
                                ps[:, 0:gcols], warm[:, 0:128], warm[:],
                                start=True, stop=True,
                            )
                    for j in range(2):
                        gq = 2 * h + j
                        nc.tensor.matmul(
                            ps[:, gcols * j:gcols * (j + 1)],
                            tbl_t[:, gq, :],
                            Ps[gq][0:CAP, :],
                            start=True,
                            stop=True,
                        )
                    dst = stage[:, 2 * gcols * h:2 * gcols * (h + 1)]
                    # PSUM->SBUF copies: gpsimd cannot touch PSUM (BIR
                    # verifier), so split ACT 3 : DVE 1. The DVE copy takes
                    # h=0, whose PSUM is ready while later P-builds of the
                    # NEXT batch are still far off -> no DVE queue blocking.
                    if h == 0:
                        nc.vector.tensor_copy(dst, ps[:])
                    else:
                        nc.scalar.copy(dst, ps[:])
                pending.append((b, stage))
                if len(pending) > 2:
                    issue_out_dmas(*pending.pop(0))
            for p in pending:
                issue_out_dmas(*p)
    nc.compile()
    return nc


def _to_bf16(x):
    return x.astype(ml_dtypes.bfloat16)


def host_prep(pillar_features, coordinates, gcols=GCOLS):
    """Per-core {table, colidx, iota_t} maps. Last write wins on dups.

    Returns None if any group's occupancy exceeds CAP (caller retries
    with smaller groups)."""
    pf = np.ascontiguousarray(np.asarray(pillar_features), dtype=np.float32)
    coords = np.asarray(coordinates)
    m = pf.shape[0]
    flat = (
        coords[:, 0].astype(np.int64) * (NY * NX)
        + coords[:, 2].astype(np.int64) * NX
        + coords[:, 3].astype(np.int64)
    )
    order = np.argsort(flat, kind="stable")
    fs = flat[order]
    is_last = np.empty(m, dtype=bool)
    if m > 1:
        is_last[:-1] = fs[:-1] != fs[1:]
    is_last[-1] = True
    occ = np.full(CELLS, -1, dtype=np.int64)
    occ[fs[is_last]] = order[is_last]

    pf_bf = _to_bf16(pf)
    ngroups = HALF // gcols
    nb = ngroups // GPB
    iota_t = np.broadcast_to(
        np.arange(gcols, dtype=np.int16), (128, gcols)
    ).copy()

    in_maps = []
    for k in range(N_CORES):
        b, yh = k // 2, k % 2
        base = b * (NY * NX) + yh * (NY // 2) * NX
        occ_k = occ[base: base + CORE_CELLS]
        p_a, p_b = occ_k[:HALF], occ_k[HALF:]

        tbl = np.zeros((nb, CAP, GPB, 128), dtype=ml_dtypes.bfloat16)
        cidx = np.full((128, ngroups), -1, dtype=np.float32)
        for g in range(ngroups):
            sl = slice(g * gcols, (g + 1) * gcols)
            ra, rb = p_a[sl], p_b[sl]
            ne = np.where((ra >= 0) | (rb >= 0))[0]
            n = len(ne)
            if n > CAP:
                return None  # caller retries with smaller groups
            bb, gq = g // GPB, g % GPB
            m_a = ra[ne] >= 0
            m_b = rb[ne] >= 0
            rows = np.zeros((n, 128), dtype=ml_dtypes.bfloat16)
            rows[m_a, :C] = pf_bf[ra[ne][m_a]]
            rows[m_b, C:] = pf_bf[rb[ne][m_b]]
            tbl[bb, :n, gq, :] = rows
            cidx[:n, g] = ne.astype(np.float32)
        in_maps.append({"table": tbl, "colidx": cidx, "iota_t": iota_t})
    return in_maps


_NC_CACHE = {}


def _get_nc(gcols):
    if gcols not in _NC_CACHE:
        _NC_CACHE[gcols] = build_nc(gcols)
    return _NC_CACHE[gcols]


def kernel(pillar_features, coordinates, batch_size):
    assert int(batch_size) == B
    from concourse.bass_utils import run_bass_kernel_spmd

    gcols = GCOLS
    in_maps = host_prep(pillar_features, coordinates, gcols)
    if in_maps is None:
        gcols = GCOLS_FB
        in_maps = host_prep(pillar_features, coordinates, gcols)
        assert in_maps is not None, "group occupancy exceeds fallback capacity"
    nc = _get_nc(gcols)
    res = run_bass_kernel_spmd(nc, in_maps, list(range(N_CORES)))

    full = np.empty((B, C, NY, NX), dtype=np.float32)
    for k in range(N_CORES):
        b, yh = k // 2, k % 2
        out_k = np.asarray(res.results[k]["out"]).astype(np.float32)
        out_k = out_k.reshape(C, NY // 2, NX)
        full[b, :, yh * (NY // 2):(yh + 1) * (NY // 2), :] = out_k
    return full


# revision 56
# speedup vs baseline: 2.4652x; 1.1532x over previous
"""PointPillarsScatter Trainium2 Bass kernel (8-core SPMD, data parallel).

Problem: scatter M=100000 pillar feature rows (C=64, fp32) into a
(B=4, C=64, NY=512, NX=512) canvas addressed by (batch, y, x)
coordinates. Duplicate coordinates resolve last-write-wins (matching
XLA CPU scatter .set; the neuron-backend reference is nondeterministic
under collisions, run-to-run noise ~1e-2 relative).

Sharding (data-parallel, no cross-core communication): core k owns
batch b = k//2 and y-half yh = k%2 — a (64, 256, 512) output slice =
131072 cells, processed as 65536 PAIR-COLUMNS (cell c, cell c+65536).

Device algorithm (one-hot matmul placement, all-bf16 data path):
the canvas is produced directly by PE matmuls out[pairpos, col] =
sum_r tbl[r, pairpos] * P[r, col], where tbl rows are the host-packed
occupied pair rows (128 bf16 = cellA ch0..63 | cellB ch0..63) of a
512-paircol group and P is the one-hot placement matrix built on DVE
via is_equal(iota, colidx[r]).  Empty columns get all-zero P columns,
so the matmul also materializes the zeros — no canvas memset, no
SBUF scatter pass, and the only DMA traffic is the packed table read
(~3.7MB/core) plus the bf16 output write (16.8MB/core), vs 45MB/core
for the memset+scatter+transpose pipeline. bf16 quantization adds
~0.2% relative error, far under the 2e-2 gate (the fp32 reference's
own scatter-collision nondeterminism is already ~0.8%).

Per core: 128 groups x 512 paircols, row capacity = the input's max
group occupancy (108 here; gcols=256 fallback recompile if it ever
exceeds 128). 16 batches x 8 groups; per batch: one ~220KB table
load, 8 P-builds (6 DVE at 4x perf mode + 2 gpsimd), 8 matmuls (PE,
bf16, 512 free columns each), 4 PSUM->SBUF f32->bf16 copies (ACT/DVE
balanced), and one gpsimd kv_writeback that moves the whole [128,
4096] stage block to DRAM at a ctx_idxs-supplied column offset. The
writeback path keeps the output entirely off the HWDGE/DMACopy
budget (its SWDGE descriptor batching is ~14x cheaper per byte in
the cost model), so the span is bounded by the balanced three-engine
compute floor (~44us each on ACT / DVE / Pool for copies, P-builds,
and writebacks) rather than by DMA bytes. All table loads are issued
up front; iota is generated on gpsimd; warm-up matmuls ride the
first PSUM tile so the PE p-state ramp (3us) completes before real
work. The output DRAM layout is [128, 65536] (partition-major halves;
the host splices channels), giving the writeback a uniform partition
stride. Modeled span ~55.7us vs 137.4us for the baseline.
"""

import sys

import numpy as np
import ml_dtypes

_TRN_REPO = "/opt/trn_rl_repo"
if _TRN_REPO not in sys.path:
    sys.path.insert(0, _TRN_REPO)

NY, NX, C, B = 512, 512, 64, 4
CELLS = B * NY * NX             # 1048576
N_CORES = 8
CORE_CELLS = CELLS // N_CORES   # 131072
HALF = CORE_CELLS // 2          # 65536 paircols per core
GCOLS = 512                     # paircols per group (one matmul)
NGROUPS = HALF // GCOLS         # 128
CAP = 112                       # row capacity per group (observed max 108)
GPB = 8                         # groups per batch
NB = NGROUPS // GPB             # 16 batches
GCOLS_FB = 256                  # fallback: smaller groups if CAP overflows


def build_nc(gcols=GCOLS, cap=CAP):
    """Build the per-core Bass program (SPMD: same NEFF on all 8 cores)."""
    from concourse import bacc, tile
    from concourse import mybir

    f32 = mybir.dt.float32
    bf16 = mybir.dt.bfloat16
    i16 = mybir.dt.int16

    ngroups = HALF // gcols
    nb = ngroups // GPB
    bcols = GPB * gcols         # paircols per batch (4096)

    nc = bacc.Bacc(
        "TRN2", target_bir_lowering=False, debug=False, num_devices=N_CORES
    )
    table = nc.dram_tensor(
        "table", [nb, cap, GPB, 128], bf16, kind="ExternalInput"
    )
    colidx = nc.dram_tensor("colidx", [128, ngroups], f32, kind="ExternalInput")
    ctxi = nc.dram_tensor("ctxi", [128, nb], mybir.dt.int32, kind="ExternalInput")
    out = nc.dram_tensor("out", [128, HALF], bf16, kind="ExternalOutput")

    with tile.TileContext(nc) as tc:
        with (
            tc.tile_pool(name="const", bufs=1) as cpool,
            tc.tile_pool(name="tbl", bufs=16) as tpool,
            tc.tile_pool(name="pmat", bufs=12) as ppool,
            tc.tile_pool(name="stage", bufs=9) as spool,
            tc.tile_pool(name="psum", bufs=4, space="PSUM") as psum_pool,
        ):
            # colidx leads the SP queue (the first P-builds wait on it);
            # ctxi rides the scalar queue; iota is built on gpsimd, off
            # the DMA budget
            colidx_sb = cpool.tile([128, ngroups], f32)
            nc.sync.dma_start(out=colidx_sb[:], in_=colidx[:])
            ctxi_sb = cpool.tile([128, nb], mybir.dt.int32)
            nc.scalar.dma_start(out=ctxi_sb[:], in_=ctxi[:])
            iota_sb = cpool.tile([128, gcols], i16)
            nc.gpsimd.iota(iota_sb[:], [[1, gcols]], channel_multiplier=0)

            # PE p-state warm-up source tile (see first-batch matmuls):
            # the ramp to full PE clock takes 3us of continuous busy. DVE
            # memsets it (DVE is free at t=0; Pool is still in preamble)
            warm = ppool.tile([128, gcols], bf16)
            nc.vector.memset(warm[:], 0.0)

            # output is written via the SWDGE kv-writeback path: one call
            # per batch moves the whole [128, 4096] stage block to out[:,
            # b*bcols : (b+1)*bcols] (ctx_idxs supplies the column offset).
            # It runs on the otherwise-idle gpsimd engine, keeps the HWDGE
            # queues free for table loads, and its descriptor batching is
            # far cheaper than an equivalent HWDGE DMACopy.
            out4 = out.rearrange("(b p) (d x) -> b p d x", b=1, d=1)

            def issue_out_wb(b, stage):
                in4 = stage[:].rearrange("p (d b x) -> p d b x", d=1, b=1)
                nc.gpsimd.kv_writeback(
                    out_ap=out4, in_ap=in4, ctx_idxs_ap=ctxi_sb[:, b:b + 1]
                )

            # all table loads issued up front: the SP queue is then pure
            # loads followed by pure output DMAs, so no output sem-wait can
            # ever delay a load dispatch, and the load stream covers the DMA
            # engines until the first outputs are copied
            tbl_ts = []
            for b in range(nb):
                tbl_t = tpool.tile([cap, GPB, 128], bf16)
                # a few early loads dispatch from the scalar queue: two
                # sequencers feeding the (shared) HWDGE beat the single-queue
                # dispatch cadence that starves the DMA engines at the start
                eng = nc.scalar if b in (1, 3) else nc.sync
                eng.dma_start(out=tbl_t[:], in_=table[b])
                tbl_ts.append(tbl_t)

            pending = []
            for b in range(nb):
                tbl_t = tbl_ts[b]
                stage = spool.tile([128, bcols], bf16)
                # all 8 P-builds up front: the single DVE copy below then
                # never head-of-line-blocks the next batch's P chain
                Ps = []
                for gq in range(GPB):
                    g = b * GPB + gq
                    P = ppool.tile([128, gcols], bf16)
                    # one-hot placement: P[r, c] = (c == colidx[r, g]).
                    # The last two builds per batch run on gpsimd (their
                    # matmuls are furthest away), offloading DVE.
                    eng = nc.gpsimd if gq >= 6 else nc.vector
                    eng.tensor_scalar(
                        P[:], iota_sb[:], colidx_sb[:, g:g + 1], None,
                        op0=mybir.AluOpType.is_equal,
                    )
                    Ps.append(P)
                for h in range(GPB // 2):
                    # 2 groups per PSUM tile (2 banks) -> 4 in-flight tiles,
                    # so matmuls never stall on a not-yet-copied PSUM bank
                    ps = psum_pool.tile([128, 2 * gcols], f32)
                    if b == 0 and h == 0:
                        # warm-up matmuls; the first real matmul's start=True
                        # PSUM reset overwrites their garbage (same engine,
                        # in-order)
                        for _ in range(6):
                            nc.tensor.matmul(
                                ps[:, 0:gcols], warm[:, 0:128], warm[:],
                                start=True, stop=True,
                            )
                    for j in range(2):
                        gq = 2 * h + j
                        nc.tensor.matmul(
                            ps[:, gcols * j:gcols * (j + 1)],
                            tbl_t[:, gq, :],
                            Ps[gq][0:cap, :],
                            start=True,
                            stop=True,
                        )
                    dst = stage[:, 2 * gcols * h:2 * gcols * (h + 1)]
                    # PSUM->SBUF copies: gpsimd cannot touch PSUM (BIR
                    # verifier), so they split ACT ~2/3 : DVE ~1/3 to
                    # balance ACT against DVE's P-build load.
                    use_dve = (h % 4 == 0 or (b % 3 == 0 and h == 2)
                               or (b in (0, nb - 1) and h % 2 == 0))
                    if use_dve:
                        nc.vector.tensor_copy(dst, ps[:])
                    else:
                        nc.scalar.copy(dst, ps[:])
                pending.append((b, stage))
                if len(pending) > 1:
                    issue_out_wb(*pending.pop(0))
            for p in pending:
                issue_out_wb(*p)
    nc.compile()
    return nc


def _to_bf16(x):
    return x.astype(ml_dtypes.bfloat16)


def host_prep(pillar_features, coordinates, gcols=GCOLS):
    """Per-core {table, colidx, iota_t} maps + row capacity (the max
    group occupancy, which sizes the table). Last write wins on dups.

    Returns None if any group's occupancy exceeds 128 rows (caller
    retries with smaller groups)."""
    pf = np.ascontiguousarray(np.asarray(pillar_features), dtype=np.float32)
    coords = np.asarray(coordinates)
    m = pf.shape[0]
    flat = (
        coords[:, 0].astype(np.int64) * (NY * NX)
        + coords[:, 2].astype(np.int64) * NX
        + coords[:, 3].astype(np.int64)
    )
    order = np.argsort(flat, kind="stable")
    fs = flat[order]
    is_last = np.empty(m, dtype=bool)
    if m > 1:
        is_last[:-1] = fs[:-1] != fs[1:]
    is_last[-1] = True
    occ = np.full(CELLS, -1, dtype=np.int64)
    occ[fs[is_last]] = order[is_last]

    pf_bf = _to_bf16(pf)
    ngroups = HALF // gcols
    nb = ngroups // GPB

    # first pass: per-core occupancy rows, to size the shared capacity
    per_core = []
    cap = 1
    for k in range(N_CORES):
        b, yh = k // 2, k % 2
        base = b * (NY * NX) + yh * (NY // 2) * NX
        occ_k = occ[base: base + CORE_CELLS]
        p_a, p_b = occ_k[:HALF], occ_k[HALF:]
        pair_occ = (p_a >= 0) | (p_b >= 0)
        gmax = int(pair_occ.reshape(ngroups, gcols).sum(axis=1).max())
        if gmax > 128:
            return None  # caller retries with smaller groups
        cap = max(cap, gmax)
        per_core.append((p_a, p_b))

    in_maps = []
    for k in range(N_CORES):
        p_a, p_b = per_core[k]

        tbl = np.zeros((nb, cap, GPB, 128), dtype=ml_dtypes.bfloat16)
        cidx = np.full((128, ngroups), -1, dtype=np.float32)
        for g in range(ngroups):
            sl = slice(g * gcols, (g + 1) * gcols)
            ra, rb = p_a[sl], p_b[sl]
            ne = np.where((ra >= 0) | (rb >= 0))[0]
            n = len(ne)
            bb, gq = g // GPB, g % GPB
            m_a = ra[ne] >= 0
            m_b = rb[ne] >= 0
            rows = np.zeros((n, 128), dtype=ml_dtypes.bfloat16)
            rows[m_a, :C] = pf_bf[ra[ne][m_a]]
            rows[m_b, C:] = pf_bf[rb[ne][m_b]]
            tbl[bb, :n, gq, :] = rows
            cidx[:n, g] = ne.astype(np.float32)
        ctxi = np.broadcast_to(
            (np.arange(nb, dtype=np.int32) * (GPB * gcols)), (128, nb)
        ).copy()
        in_maps.append({"table": tbl, "colidx": cidx, "ctxi": ctxi})
    return in_maps, cap


_NC_CACHE = {}


def _get_nc(gcols, cap):
    if (gcols, cap) not in _NC_CACHE:
        _NC_CACHE[(gcols, cap)] = build_nc(gcols, cap)
    return _NC_CACHE[(gcols, cap)]


def kernel(pillar_features, coordinates, batch_size):
    assert int(batch_size) == B
    from concourse.bass_utils import run_bass_kernel_spmd

    gcols = GCOLS
    prep = host_prep(pillar_features, coordinates, gcols)
    if prep is None:
        gcols = GCOLS_FB
        prep = host_prep(pillar_features, coordinates, gcols)
        assert prep is not None, "group occupancy exceeds fallback capacity"
    in_maps, cap = prep
    nc = _get_nc(gcols, cap)
    res = run_bass_kernel_spmd(nc, in_maps, list(range(N_CORES)))

    full = np.empty((B, C, NY, NX), dtype=np.float32)
    for k in range(N_CORES):
        b, yh = k // 2, k % 2
        o = np.asarray(res.results[k]["out"]).astype(np.float32)
        # row p<64 = channel p of cells [0, HALF); p>=64 = channel p-64
        # of cells [HALF, 2*HALF)
        out_k = np.concatenate([o[0:C], o[C:2 * C]], axis=1)
        out_k = out_k.reshape(C, NY // 2, NX)
        full[b, :, yh * (NY // 2):(yh + 1) * (NY // 2), :] = out_k
    return full

